# revision 6
# baseline (speedup 1.0000x reference)
"""Trainium2 Bass kernel for nn_LocalRouter (sparse_attention).

Computation (reference semantics):
  local:  h_w = silu(mu_n @ Wm1_top + mu_{n-w} @ Wm1_bot + bm1), w=1..4
          local = mean_w(h_w) @ Wm2 + bm2
  global: scores = (mu @ Wq) @ (mu @ Wk)^T / sqrt(D), causal; top-8 -> softmax
          global = probs @ mu @ Wv + bv        (rows of probs sum to 1)
  out = concat([local, global]) @ Wo + bo

Algebraic refactors (host-side weight fusion, exact in fp32):
  scores = (mu @ Wqks) @ mu^T, Wqks = Wq @ Wk^T / sqrt(D)   [bk shifts a row
      uniformly -> no-op through top-k+softmax; bq term vanishes for bq==0]
  out = hbar @ Wmo + gsum @ Wvo + bconst
      hbar = sum_w silu(...), Wmo = (Wm2 @ Wo_top)/4, Wvo = Wv @ Wo_bot,
      gsum = sum_k p_k mu[idx_k], bconst = bo + bm2 @ Wo_top + bv @ Wo_bot

Sharding: core c -> batch b=c//2, half h=c%2 owns query tiles {t: t%2==h}
(interleaved for causal load balance). Scores run in PE fp32 (bf16 scores
flip top-8 selection on ~4% of rows; one flip costs ~0.11 absmax). The rest
runs bf16.
"""

import math
import numpy as np
import ml_dtypes

B, N, D = 4, 4096, 512
WIN, TOPK = 4, 8
P = 128
NCORES = 8
NSLOT = 16            # query tiles owned per core
NEG = -1.0e30

_cache = {}


def _build_program():
    """Build the (core-uniform) Bass program once. Returns the compiled Bacc."""
    if "nc" in _cache:
        return _cache["nc"]
    from contextlib import ExitStack
    import concourse.bass as bass
    import concourse.tile as tile
    import concourse.mybir as mybir
    from concourse import bacc
    from concourse.bass import IndirectOffsetOnAxis
    from concourse.masks import make_identity

    dt = mybir.dt
    AF = mybir.ActivationFunctionType
    OP = mybir.AluOpType

    nc = bacc.Bacc(
        "TRN2",
        target_bir_lowering=False,
        debug=False,
        enable_asserts=False,
        num_devices=NCORES,
    )

    f32, b16 = dt.float32, dt.bfloat16
    # ---- DRAM I/O (per-core data; program identical on all cores) ----
    muT = nc.dram_tensor("muT", [4, P, N], f32, kind="ExternalInput").ap()
    muq = nc.dram_tensor("muq", [4, P, NSLOT * P], f32, kind="ExternalInput").ap()
    muloc = nc.dram_tensor("muloc", [4, P, NSLOT * 132], b16, kind="ExternalInput").ap()
    mukeys = nc.dram_tensor("mukeys", [N, D], f32, kind="ExternalInput").ap()
    wqks = nc.dram_tensor("wqks", [4, P, D], f32, kind="ExternalInput").ap()
    wm1t = nc.dram_tensor("wm1t", [4, P, D], b16, kind="ExternalInput").ap()
    wm1b = nc.dram_tensor("wm1b", [4, P, D], b16, kind="ExternalInput").ap()
    wmo = nc.dram_tensor("wmo", [4, P, D], b16, kind="ExternalInput").ap()
    wvo = nc.dram_tensor("wvo", [4, P, D], b16, kind="ExternalInput").ap()
    trimask = nc.dram_tensor("trimask", [P, 256], f32, kind="ExternalInput").ap()
    bm1t = nc.dram_tensor("bm1t", [P, 4], f32, kind="ExternalInput").ap()
    bconst = nc.dram_tensor("bconst", [P, 4], f32, kind="ExternalInput").ap()
    outT = nc.dram_tensor("outT", [4, P, NSLOT * P], f32, kind="ExternalOutput").ap()
    # scratch for the index-layout roundtrip (topk idx -> wrapped int16)
    iw_dram = nc.dram_tensor("iw_dram", [NSLOT, P, TOPK], dt.uint32,
                             kind="Internal").ap()

    with tile.TileContext(nc) as tc, ExitStack() as ctx:
        consts = ctx.enter_context(tc.tile_pool(name="consts", bufs=1))
        qpool = ctx.enter_context(tc.tile_pool(name="qpool", bufs=2))
        strip_pool = ctx.enter_context(tc.tile_pool(name="strip", bufs=1))
        top_pool = ctx.enter_context(tc.tile_pool(name="top", bufs=2))
        gpool = ctx.enter_context(tc.tile_pool(name="gather", bufs=1))
        acc_pool = ctx.enter_context(tc.tile_pool(name="acc", bufs=2))
        gt_pool = ctx.enter_context(tc.tile_pool(name="globT", bufs=2))
        loc_pool = ctx.enter_context(tc.tile_pool(name="loc", bufs=2))
        hbar_pool = ctx.enter_context(tc.tile_pool(name="hbar", bufs=1))
        out_pool = ctx.enter_context(tc.tile_pool(name="outstage", bufs=2))

        ps_score = ctx.enter_context(tc.tile_pool(name="ps_score", bufs=2, space="PSUM"))
        ps_qh = ctx.enter_context(tc.tile_pool(name="ps_qh", bufs=1, space="PSUM"))
        ps_tp = ctx.enter_context(tc.tile_pool(name="ps_tp", bufs=1, space="PSUM"))
        ps_a = ctx.enter_context(tc.tile_pool(name="ps_a", bufs=1, space="PSUM"))
        ps_b = ctx.enter_context(tc.tile_pool(name="ps_b", bufs=2, space="PSUM"))
        ps_o = ctx.enter_context(tc.tile_pool(name="ps_o", bufs=1, space="PSUM"))

        # ---- resident constants ----
        muT_sb = consts.tile([P, 4, N], f32)
        for di in range(4):
            nc.sync.dma_start(muT_sb[:, di, :], muT[di])
        wqks_sb = consts.tile([P, 4, D], f32)
        wm1t_sb = consts.tile([P, 4, D], b16)
        wm1b_sb = consts.tile([P, 4, D], b16)
        wmo_sb = consts.tile([P, 4, D], b16)
        wvo_sb = consts.tile([P, 4, D], b16)
        for sb, dr in ((wqks_sb, wqks), (wm1t_sb, wm1t), (wm1b_sb, wm1b),
                       (wmo_sb, wmo), (wvo_sb, wvo)):
            for di in range(4):
                nc.sync.dma_start(sb[:, di, :], dr[di])
        trimask_sb = consts.tile([P, 256], f32)
        nc.sync.dma_start(trimask_sb[:], trimask[:])
        bm1t_sb = consts.tile([P, 4], f32)
        nc.sync.dma_start(bm1t_sb[:], bm1t[:])
        bconst_sb = consts.tile([P, 4], f32)
        nc.sync.dma_start(bconst_sb[:], bconst[:])
        ident = consts.tile([P, P], f32)
        make_identity(nc, ident[:])
        hbar = hbar_pool.tile([P, 4, NSLOT * P], b16)

        def emit_slot(s, globalT):
            """Query tile slot s: scores (fp32) -> top8 -> softmax -> gather
            -> weighted sum -> transpose into globalT[:, :, (s%4)*128:...]."""
            KR = 256 * (s + 1)  # key range [0, KR)
            # qhT for this slot: psum[do_part, 4 do_tile, 128 q]
            mq = qpool.tile([P, 4, P], f32, tag="muq")
            nc.sync.dma_start(
                mq[:], muq[:, :, s * P:(s + 1) * P].rearrange("a p c -> p a c"))
            qh_ps = ps_qh.tile([P, 4, P], f32)
            for do in range(4):
                for di in range(4):
                    nc.tensor.matmul(
                        qh_ps[:, do, :],
                        wqks_sb[:, di, do * P:(do + 1) * P],
                        mq[:, di, :],
                        start=(di == 0), stop=(di == 3))
            qh = qpool.tile([P, 4, P], f32, tag="qh")
            for do in range(4):
                nc.scalar.copy(qh[:, do, :], qh_ps[:, do, :])

            # scores strip [128 q, KR keys] fp32
            strip = strip_pool.tile([P, N], f32, tag="strip")
            nchunks = (KR + 511) // 512
            for c in range(nchunks):
                k0 = c * 512
                csz = min(512, KR - k0)
                sps = ps_score.tile([P, 512], f32, tag="sps")
                for di in range(4):
                    nc.tensor.matmul(
                        sps[:, :csz],
                        qh[:, di, :],
                        muT_sb[:, di, k0:k0 + csz],
                        start=(di == 0), stop=(di == 3))
                nc.scalar.copy(strip[:, k0:k0 + csz], sps[:, :csz])
            # causal mask over the last 256 keys (handles diagonal + padding)
            nc.vector.tensor_tensor(
                strip[:, KR - 256:KR], strip[:, KR - 256:KR], trimask_sb[:],
                op=OP.add)

            # top-8 values + indices
            v8 = top_pool.tile([P, TOPK], f32, tag="v8")
            nc.vector.max(out=v8[:], in_=strip[:, :KR])
            i8 = top_pool.tile([P, TOPK], dt.uint32, tag="i8")
            nc.vector.max_index(out=i8[:], in_max=v8[:], in_values=strip[:, :KR])
            # softmax over the 8 (masked entries are ~-1e30 -> exp ~ 0)
            nmax = top_pool.tile([P, 1], f32, tag="nmax")
            nc.vector.tensor_scalar_mul(nmax[:], v8[:, 0:1], -1.0)
            e8 = top_pool.tile([P, TOPK], f32, tag="e8")
            zsum = top_pool.tile([P, 1], f32, tag="zsum")
            nc.scalar.activation(e8[:], v8[:], AF.Exp, bias=nmax[:],
                                 accum_out=zsum[:])
            zr = top_pool.tile([P, 1], f32, tag="zr")
            nc.vector.reciprocal(zr[:], zsum[:])
            p8 = top_pool.tile([P, TOPK], f32, tag="p8")
            nc.vector.tensor_scalar_mul(p8[:], e8[:], zr[:])

            # gather the 8 mu rows per query from DRAM via dma_gather.
            # Index layout: wrapped[(k*128+p)%16, (k*128+p)//16] = i8[p, k],
            # built with a DRAM roundtrip + 8 replicate loads.
            nc.sync.dma_start(iw_dram[s], i8[:])
            iw32 = top_pool.tile([P, TOPK * P // 16], dt.uint32, tag="iw32")
            flat = iw_dram[s].rearrange("a b -> (a b)")
            for rep in range(8):
                src_ap = bass.AP(flat.tensor, flat.offset,
                                 [[8, 16], [1, TOPK], [TOPK * 16, 8]])
                nc.sync.dma_start(
                    iw32[16 * rep:16 * rep + 16, :]
                    .rearrange("p (k h) -> p k h", k=TOPK), src_ap)
            iw = top_pool.tile([P, TOPK * P // 16], dt.int16, tag="iw")
            nc.vector.tensor_copy(iw[:], iw32[:])
            g = gpool.tile([P, TOPK, D], f32, tag="g")
            nc.gpsimd.dma_gather(g[:], mukeys[:], iw[:], num_idxs=TOPK * P,
                                 num_idxs_reg=TOPK * P, elem_size=D)
            # weighted sum: gsum[q, :] = sum_k p8[q,k] * g[q,k,:]
            acc = acc_pool.tile([P, D], f32, tag="acc")
            nc.vector.tensor_scalar_mul(acc[:], g[:, 0, :], p8[:, 0:1])
            for k in range(1, TOPK):
                nc.vector.scalar_tensor_tensor(
                    acc[:], g[:, k, :], p8[:, k:k + 1], acc[:],
                    op0=OP.mult, op1=OP.add)
            # transpose to [d, q] (bf16) for the output projection
            for j in range(4):
                tp = ps_tp.tile([P, P], f32, tag="tp")
                nc.tensor.transpose(tp[:], acc[:, j * P:(j + 1) * P], ident[:])
                nc.scalar.copy(globalT[:, j, (s % 4) * P:(s % 4 + 1) * P], tp[:])

        def emit_local(grp):
            """Local branch for own-tile group grp (4 query tiles, 512 rows):
            hbar[:, :, grp*512:...] = sum_w silu(A + B_shift(w) + bm1)."""
            r0 = grp * 512
            mub = loc_pool.tile([P, 4, 4, 132], b16, tag="mub")
            nc.sync.dma_start(
                mub[:], muloc[:, :, grp * 528:(grp + 1) * 528]
                .rearrange("a p (t c) -> p a t c", c=132))
            for dh in range(4):
                a_ps = ps_a.tile([P, 512], f32, tag="a_ps")
                for di in range(4):
                    nc.tensor.matmul(
                        a_ps[:],
                        wm1t_sb[:, di, dh * P:(dh + 1) * P],
                        mub[:, di, :, 4:132],
                        start=(di == 0), stop=(di == 3))
                bsb = loc_pool.tile([P, 2, 264], b16, tag="bsb")
                for half in range(2):
                    b_ps = ps_b.tile([P, 264], f32, tag="b_ps")
                    for di in range(4):
                        nc.tensor.matmul(
                            b_ps[:],
                            wm1b_sb[:, di, dh * P:(dh + 1) * P],
                            mub[:, di, 2 * half:2 * half + 2, :],
                            start=(di == 0), stop=(di == 3))
                    nc.scalar.copy(bsb[:, half, :], b_ps[:])
                bsh = bsb[:].rearrange("p a (t c) -> p a t c", c=132)
                hs = hbar[:, dh, r0:r0 + 512]
                # silu(x) = x * sigmoid(x); x = A + bm1 + B_shift(w)
                for w in range(1, WIN + 1):
                    x = loc_pool.tile([P, 512], b16, tag="xw")
                    nc.vector.scalar_tensor_tensor(
                        x[:], a_ps[:], bm1t_sb[:, dh:dh + 1],
                        bsh[:, :, :, 4 - w:4 - w + 128],
                        op0=OP.add, op1=OP.add)
                    sg = loc_pool.tile([P, 512], b16, tag="sg")
                    nc.scalar.activation(sg[:], x[:], AF.Sigmoid)
                    if w == 1:
                        nc.vector.tensor_tensor(hs, x[:], sg[:], op=OP.mult)
                    else:
                        wt = loc_pool.tile([P, 512], b16, tag="wt")
                        nc.vector.tensor_tensor(wt[:], x[:], sg[:], op=OP.mult)
                        nc.vector.tensor_tensor(hs, hs, wt[:], op=OP.add)

        def emit_outproj(grp, globalT):
            r0 = grp * 512
            for do in range(4):
                o_ps = ps_o.tile([P, 512], f32, tag="o_ps")
                for dm in range(4):
                    nc.tensor.matmul(
                        o_ps[:],
                        wmo_sb[:, dm, do * P:(do + 1) * P],
                        hbar[:, dm, r0:r0 + 512],
                        start=(dm == 0), stop=False)
                for dm in range(4):
                    nc.tensor.matmul(
                        o_ps[:],
                        wvo_sb[:, dm, do * P:(do + 1) * P],
                        globalT[:, dm, :],
                        start=False, stop=(dm == 3))
                ost = out_pool.tile([P, 512], f32, tag="ost")
                nc.scalar.activation(ost[:], o_ps[:], AF.Identity,
                                     bias=bconst_sb[:, do:do + 1])
                nc.sync.dma_start(outT[do, :, r0:r0 + 512], ost[:])

        for grp in range(4):
            globalT = gt_pool.tile([P, 4, 512], b16, tag="globalT")
            for s in range(4 * grp, 4 * grp + 4):
                emit_slot(s, globalT)
            emit_local(grp)
            emit_outproj(grp, globalT)

    nc.compile()
    _cache["nc"] = nc
    return nc


def _prep_core_inputs(c, mu, Wqks, Wm1, bm1_, consts):
    """Host-side sharding/layout for core c."""
    f32 = np.float32
    b16 = ml_dtypes.bfloat16
    b, h = c // 2, c % 2
    mub = np.ascontiguousarray(mu[b])                       # [N, D] f32
    t_own = list(range(h, 32, 2))
    own = np.concatenate([mub[128 * t:128 * t + 128] for t in t_own])  # [2048, D]
    muT = np.ascontiguousarray(mub.T).reshape(4, P, N).astype(f32)
    muq = np.ascontiguousarray(own.T).reshape(4, P, NSLOT * P).astype(f32)
    # local strips: per own tile, rows [128t-4, 128t+128) zero-padded at n<0
    strips = []
    for t in t_own:
        st = np.zeros((132, D), f32)
        lo = 128 * t - 4
        src_lo = max(lo, 0)
        st[src_lo - lo:] = mub[src_lo:128 * t + 128]
        strips.append(st)
    muloc = np.concatenate(strips)                          # [2112, D]
    muloc = np.ascontiguousarray(muloc.T).reshape(4, P, NSLOT * 132).astype(b16)
    # causal tri-mask on the last 256 keys of each slot strip
    tm = np.zeros((P, 256), f32)
    j = np.arange(128)[None, :]
    p = np.arange(128)[:, None]
    if h == 0:
        tm[:, :128] = np.where(j <= p, 0.0, NEG)
        tm[:, 128:] = NEG
    else:
        tm[:, 128:] = np.where(j <= p, 0.0, NEG)
    return dict(
        muT=muT, muq=muq, muloc=muloc,
        mukeys=mub.astype(f32),
        trimask=tm,
        **consts,
    )


def prep_in_maps(inputs):
    mu = np.asarray(inputs["mu"], np.float32)
    Wq = np.asarray(inputs["Wq"], np.float32)
    bq = np.asarray(inputs["bq"], np.float32)
    Wk = np.asarray(inputs["Wk"], np.float32)
    Wv = np.asarray(inputs["Wv"], np.float32)
    bv = np.asarray(inputs["bv"], np.float32)
    Wm1 = np.asarray(inputs["Wm1"], np.float32)
    bm1 = np.asarray(inputs["bm1"], np.float32)
    Wm2 = np.asarray(inputs["Wm2"], np.float32)
    bm2 = np.asarray(inputs["bm2"], np.float32)
    Wo = np.asarray(inputs["Wo"], np.float32)
    bo = np.asarray(inputs["bo"], np.float32)
    assert not bq.any(), "bq != 0 unsupported (adds a per-key score term)"

    f32 = np.float32
    b16 = ml_dtypes.bfloat16
    Wqks = (Wq @ Wk.T / math.sqrt(D)).astype(f32)
    Wmo = ((Wm2 @ Wo[:D]) / WIN).astype(f32)
    Wvo = (Wv @ Wo[D:]).astype(f32)
    bconst = (bo + bm2 @ Wo[:D] + bv @ Wo[D:]).astype(f32)
    consts = dict(
        wqks=Wqks.reshape(4, P, D),
        wm1t=np.ascontiguousarray(Wm1[:D]).reshape(4, P, D).astype(b16),
        wm1b=np.ascontiguousarray(Wm1[D:]).reshape(4, P, D).astype(b16),
        wmo=Wmo.reshape(4, P, D).astype(b16),
        wvo=Wvo.reshape(4, P, D).astype(b16),
        bm1t=np.ascontiguousarray(bm1.reshape(4, P).T),
        bconst=np.ascontiguousarray(bconst.reshape(4, P).T),
    )
    return [_prep_core_inputs(c, mu, Wqks, Wm1, bm1, consts)
            for c in range(NCORES)]


def assemble(core_outs):
    """core_outs: list of outT arrays [4, P, 2048] per core -> full [B, N, D]."""
    out = np.empty((B, N, D), np.float32)
    for c in range(NCORES):
        b, h = c // 2, c % 2
        oT = np.asarray(core_outs[c])
        oc = np.ascontiguousarray(oT.reshape(D, NSLOT * P).T)  # [2048, D]
        for s, t in enumerate(range(h, 32, 2)):
            out[b, 128 * t:128 * t + 128] = oc[128 * s:128 * s + 128]
    return out


def kernel(**inputs):
    nc = _build_program()
    in_maps = prep_in_maps(inputs)

    import os
    from concourse.bass_utils import run_bass_kernel_spmd
    trace = bool(int(os.environ.get("LR_TRACE", "0")))
    res = run_bass_kernel_spmd(nc, in_maps, core_ids=list(range(NCORES)),
                               trace=trace)
    _cache["last_results"] = res
    return assemble([res.results[c]["outT"] for c in range(NCORES)])
